# revision 4
# baseline (speedup 1.0000x reference)
"""FlowNetC-style SpatialCorrelationSampler (max_disp=20, dilation_patch=2)
as a Bass/Tile kernel for 8 Trainium2 NeuronCores.

Strategy (v3)
-------------
Data-parallel over batch: core i handles sample i (B == 8 == n_cores).

Per core the even displacements split the problem into 4 phase
sub-problems over the (y%2, x%2) sub-grids (32 x 48), each with
sub-displacements in [-10, 10]^2.  Phase images use a zero-padded
row pitch of 58 (flat position n = 58*y + x + 10, NP = 1866), so the
TensorEngine Gram matrix G[m, n] = sum_c x2p[c, m] * x1p[c, n] holds
every output as a diagonal: out[s, n] = G[n + s - S0, n].

v3 changes vs the first working kernel (267 us):
- Extraction matmuls are merged across sigma windows: one desc per
  (n0, t, sig-wave) streams a [sig x k] rectangle through a single
  shifted-identity stationary E[:, a:a+128].  The slab is padded with
  zero chunks below/above so out-of-range (sig, k) cells read exact
  zeros instead of needing per-sigma window clipping.  Desc count
  drops ~6200 -> ~4100 and each desc's moving operand is ~3x wider,
  so the PE no longer pays the per-instruction floor.
- Extraction psum blocks are packed per n0 (only the union k-window),
  sig-blocks at a 512-f32 (one bank) stride; one desc writes 3 banks.
- Output tensor is bf16 (host casts to f32): halves the out-DMA bytes
  and the ob staging SBUF.
- Gram runs px-split with the slab holding one px phase at a time.
"""

import sys

for _p in ("/opt/trn_rl_repo",):
    if _p not in sys.path:
        sys.path.insert(0, _p)

import numpy as np

import concourse.bass as bass
import concourse.tile as tile
from concourse import bacc, mybir
from concourse.bass_utils import run_bass_kernel_spmd

F32 = mybir.dt.float32
BF16 = mybir.dt.bfloat16

B = 8
C = 256
H = 64
W = 96
HS, WS = H // 2, W // 2      # 32 x 48 sub-grid
R = 10                       # sub-displacement radius
P = 2 * R + 1                # 21
D = P * P                    # 441 output channels
WP = WS + R                  # 58: padded row pitch
XOFF = R
NP = WP * (HS - 1) + WS + 2 * R  # 1866 flat positions
NK = 15                      # 128-row m-chunks covering [0, 1866)
S0 = WP * R + XOFF           # 590: s_idx = (m - n) + S0
SIDX = WP * (P - 1) + P      # 1181 used diagonal offsets
NSIG = (SIDX + 127) // 128   # 10 sigma windows
KSTEP = NP + 128             # staircase step between k-chunk diagonals

WAVES = [(0, 3), (3, 6), (6, 9), (9, 10)]   # sigma waves
GROUPS = [(i * 16, (i + 1) * 16) for i in range(8)]   # n0 groups
BS = 64                      # psum block stride per n0 (f32), divides 512


def _t_list(n0):
    return [t for t in range(0, 12) if -128 < 128 * t + n0 - S0 < 128]


def _kmax(n0):
    return NK if n0 < NP - 128 * (NK - 1) else NK - 1


def _valid_win(sig, t, n0):
    lo, hi = max(0, t - sig), min(_kmax(n0), NK - (sig - t))
    return (lo, hi) if lo < hi else None


def _union_win(n0, s0, s1):
    """Union k-window over t in t_list and sig in [s0, s1)."""
    lo, hi = None, None
    for t in _t_list(n0):
        for sig in range(s0, s1):
            w = _valid_win(sig, t, n0)
            if w:
                lo = w[0] if lo is None else min(lo, w[0])
                hi = w[1] if hi is None else max(hi, w[1])
    return (lo, hi) if lo is not None else None


def _regime_runs(q0, q1):
    """n0 runs inside [q0,q1) where (t_list, kmax) is constant."""
    runs, start = [], q0
    for n0 in range(q0 + 1, q1 + 1):
        if n0 == q1 or (_t_list(n0), _kmax(n0)) != (_t_list(start), _kmax(start)):
            runs.append((start, n0))
            start = n0
    return runs


def _pads():
    lo = hi = 0
    for (s0, s1) in WAVES:
        for n0 in range(128):
            u = _union_win(n0, s0, s1)
            if not u:
                continue
            for t in _t_list(n0):
                lo = max(lo, t - s0 - u[0])
                hi = max(hi, (s1 - 1) - t + u[1] - 1 - (NK - 1))
    return max(lo, 0), max(hi, 0)


PADLO, PADHI = _pads()
NCH = PADLO + NK + PADHI     # total slab chunks


def _build_program(loop_n=None):
    nc = bacc.Bacc("TRN2", target_bir_lowering=False, debug=False)

    x1d = nc.dram_tensor("x1", [C, H, W], F32, kind="ExternalInput").ap()
    x2d = nc.dram_tensor("x2", [C, H, W], F32, kind="ExternalInput").ap()
    outd = nc.dram_tensor("out", [D, H, W], BF16, kind="ExternalOutput").ap()

    with tile.TileContext(nc) as tc:
        if loop_n is None:
            _corr_kernel(tc, outd, x1d, x2d)
        else:
            with tc.For_i(0, loop_n, 1):
                _corr_kernel(tc, outd, x1d, x2d)
    nc.compile()
    return nc


def _corr_kernel(tc, outd, x1d, x2d):
    from contextlib import ExitStack

    nc = tc.nc
    with ExitStack() as ctx:
        const_pool = ctx.enter_context(tc.tile_pool(name="const", bufs=1))
        stage_pool = ctx.enter_context(tc.tile_pool(name="stage", bufs=1))
        pack_pool = ctx.enter_context(tc.tile_pool(name="pack", bufs=1))
        slab_pool = ctx.enter_context(tc.tile_pool(name="slab", bufs=1))
        outb_pool = ctx.enter_context(tc.tile_pool(name="outb", bufs=1))
        psum_pool = ctx.enter_context(
            tc.tile_pool(name="psum", bufs=1, space="PSUM"))

        # --- shifted-identity strip: E[p, f] = 1 iff f - p == 128 ---
        ones = stage_pool.tile([128, 384], BF16, tag="stage")
        nc.vector.memset(ones[:, :], 1.0)
        E = const_pool.tile([128, 384], BF16)
        nc.gpsimd.affine_select(
            E[:, :], ones[:, :], pattern=[[1, 384]], base=-128,
            channel_multiplier=-1,
            compare_op=mybir.AluOpType.is_equal, fill=0.0)

        # --- persistent slab with zero pad chunks (memset once) ---
        slab = slab_pool.tile([128, NCH * NP], BF16, tag="slab")
        rp_s = slab.ap[0][0]
        if PADLO:
            nc.vector.memset(slab[:, 0:PADLO * NP], 0.0)
        if PADHI:
            nc.vector.memset(
                slab[:, (PADLO + NK) * NP:NCH * NP], 0.0)

        n_bal = 0

        def copy_op(dst, src):
            nonlocal n_bal
            if n_bal % 2:
                nc.scalar.copy(dst, src)
            else:
                nc.vector.tensor_copy(dst, src)
            n_bal += 1

        # ob tiles: one per sigma, px-interleaved cols 2n+px, bf16
        obs = []
        for sig in range(NSIG):
            obs.append(outb_pool.tile(
                [128, 2 * (NK * 128)], BF16, tag=f"ob{sig}",
                name=f"ob_{sig}"))

        for py in (0, 1):
            # ---- load + phase-pack inputs (both px) ----
            packed = {}
            for t_idx, src in ((0, x1d), (1, x2d)):
                for cc in (0, 1):
                    stg = stage_pool.tile([128, HS * W], F32, tag="stage",
                                          name=f"stg_{py}_{t_idx}_{cc}")
                    nc.sync.dma_start(
                        stg.rearrange("p (y x) -> p y x", x=W),
                        src[cc * 128:(cc + 1) * 128, py::2, :])
                    stg_v = stg.rearrange("p (y x) -> p y x", x=W)
                    for px in (0, 1):
                        pk = pack_pool.tile(
                            [128, NK * 128], BF16, tag=f"pk{t_idx}{cc}{px}",
                            name=f"pk_{py}_{t_idx}_{cc}_{px}")
                        nc.vector.memset(pk[:, :], 0.0)
                        dstv = bass.AP(pk.tensor, XOFF,
                                       [[pk.ap[0][0], 128], [WP, HS], [1, WS]])
                        copy_op(dstv, stg_v[:, :, px::2])
                        packed[(t_idx, cc, px)] = pk

            for px in (0, 1):
                # ---- Gram into padded slab (chunks PADLO..PADLO+NK) ----
                for k in range(NK):
                    for nb in range(4):
                        nbo = nb * 512
                        nbw = min(512, NP - nbo)
                        ps = psum_pool.tile([128, 512], F32,
                                            tag=f"g{(k * 4 + nb) % 2}",
                                            name=f"psg_{py}_{px}_{k}_{nb}")
                        for cc in (0, 1):
                            nc.tensor.matmul(
                                ps[:, 0:nbw],
                                lhsT=packed[(1, cc, px)][:, 128 * k:128 * (k + 1)],
                                rhs=packed[(0, cc, px)][:, nbo:nbo + nbw],
                                start=(cc == 0), stop=(cc == 1))
                        base = (PADLO + k) * NP + nbo
                        copy_op(slab[:, base:base + nbw], ps[:, 0:nbw])

                # ---- extraction: sig-wave x n0-group tiles ----
                # desc walk is k-outer / sig-inner (PE moving AP strides
                # must decrease outer->inner); psum block per n0 is a
                # contiguous [kw x ns] brick at a BS stride (bank-safe).
                for wi, (s0, s1) in enumerate(WAVES):
                    ns = s1 - s0
                    for gi, (q0, q1) in enumerate(GROUPS):
                        ps = psum_pool.tile(
                            [128, 16 * BS], F32, tag=f"ex{(gi + 8 * wi) % 3}",
                            name=f"pse_{py}_{px}_{wi}_{gi}")
                        pp = ps.ap[0][0]
                        # descs
                        for n0 in range(q0, q1):
                            u = _union_win(n0, s0, s1)
                            if not u:
                                continue
                            klo, khi = u
                            kw = khi - klo
                            ts = [t for t in _t_list(n0)
                                  if any(_valid_win(sig, t, n0)
                                         for sig in range(s0, s1))]
                            for ti, t in enumerate(ts):
                                a = 128 * t + n0 - S0 + 128
                                rhs = bass.AP(
                                    slab.tensor,
                                    NP * (PADLO + s0 - t + klo)
                                    + n0 + 128 * klo,
                                    [[rp_s, 128], [KSTEP, kw], [NP, ns]])
                                dst = bass.AP(
                                    ps.tensor, (n0 - q0) * BS,
                                    [[pp, 128], [ns, kw], [1, ns]])
                                nc.tensor.matmul(
                                    dst, lhsT=E[:, a:a + 128], rhs=rhs,
                                    start=(ti == 0), stop=(ti == len(ts) - 1))
                        # evac per (sig, regime-run)
                        for sl in range(ns):
                            sig = s0 + sl
                            for (r0, r1) in _regime_runs(q0, q1):
                                u = _union_win(r0, s0, s1)
                                if not u:
                                    continue
                                klo, khi = u
                                kw = khi - klo
                                src = bass.AP(
                                    ps.tensor, (r0 - q0) * BS + sl,
                                    [[pp, 128], [BS, r1 - r0], [ns, kw]])
                                dstv = bass.AP(
                                    obs[sig].tensor,
                                    2 * (r0 + 128 * klo) + px,
                                    [[obs[sig].ap[0][0], 128],
                                     [2, r1 - r0], [256, kw]])
                                copy_op(dstv, src)

            # ---- stale zero + DMA out per sigma ----
            for (s0, s1) in WAVES:
                for sig in range(s0, s1):
                    ob = obs[sig]
                    rp_o = ob.ap[0][0]
                    # zero ob columns never evac'd (out-of-range dy)
                    for (r0, r1) in _regime_runs(0, 128):
                        u = _union_win(r0, s0, s1)
                        km = _kmax(r0)
                        stales = []
                        if not u:
                            stales.append((0, km))
                        else:
                            if u[0] > 0:
                                stales.append((0, u[0]))
                            if u[1] < km:
                                stales.append((u[1], km))
                        for (ka, kb) in stales:
                            zap = bass.AP(
                                ob.tensor, 2 * (r0 + 128 * ka),
                                [[rp_o, 128], [2, r1 - r0],
                                 [256, kb - ka], [1, 2]])
                            nc.gpsimd.memset(zap, 0.0)
                    # d-contiguous row runs
                    sig_lo = 128 * sig
                    sig_hi = min(sig_lo + 128, SIDX)
                    for di in range(P):
                        r_lo = max(WP * di, sig_lo)
                        r_hi = min(WP * di + P, sig_hi)
                        if r_lo >= r_hi:
                            continue
                        dj0, dj1 = r_lo - WP * di, r_hi - WP * di
                        d0 = P * di + dj0
                        src = bass.AP(
                            ob.tensor,
                            (r_lo - sig_lo) * rp_o + 2 * XOFF,
                            [[rp_o, r_hi - r_lo], [2 * WP, HS], [1, W]])
                        nc.sync.dma_start(
                            outd[d0:d0 + (dj1 - dj0), py::2, :], src)


_PROGRAM = None


def _get_program():
    global _PROGRAM
    if _PROGRAM is None:
        _PROGRAM = _build_program()
    return _PROGRAM


def kernel(x1: np.ndarray, x2: np.ndarray) -> np.ndarray:
    x1 = np.ascontiguousarray(np.asarray(x1, dtype=np.float32))
    x2 = np.ascontiguousarray(np.asarray(x2, dtype=np.float32))
    assert x1.shape == (B, C, H, W) and x2.shape == (B, C, H, W)
    nc = _get_program()
    in_maps = [{"x1": x1[i], "x2": x2[i]} for i in range(B)]
    res = run_bass_kernel_spmd(nc, in_maps, core_ids=list(range(B)))
    return np.stack(
        [np.asarray(res.results[i]["out"]).astype(np.float32)
         for i in range(B)], axis=0)


if __name__ == "__main__":
    rng = np.random.default_rng(0)
    x1 = rng.standard_normal((B, C, H, W), dtype=np.float32)
    x2 = rng.standard_normal((B, C, H, W), dtype=np.float32)
    out = kernel(x1, x2)
    print(out.shape, out.dtype, float(np.abs(out).max()))


# revision 5
# speedup vs baseline: 1.0077x; 1.0077x over previous
"""FlowNetC-style SpatialCorrelationSampler (max_disp=20, dilation_patch=2)
as a Bass/Tile kernel for 8 Trainium2 NeuronCores.

Strategy (v3)
-------------
Data-parallel over batch: core i handles sample i (B == 8 == n_cores).

Per core the even displacements split the problem into 4 phase
sub-problems over the (y%2, x%2) sub-grids (32 x 48), each with
sub-displacements in [-10, 10]^2.  Phase images use a zero-padded
row pitch of 58 (flat position n = 58*y + x + 10, NP = 1866), so the
TensorEngine Gram matrix G[m, n] = sum_c x2p[c, m] * x1p[c, n] holds
every output as a diagonal: out[s, n] = G[n + s - S0, n].

v3 changes vs the first working kernel (267 us):
- Extraction matmuls are merged across sigma windows: one desc per
  (n0, t, sig-wave) streams a [sig x k] rectangle through a single
  shifted-identity stationary E[:, a:a+128].  The slab is padded with
  zero chunks below/above so out-of-range (sig, k) cells read exact
  zeros instead of needing per-sigma window clipping.  Desc count
  drops ~6200 -> ~4100 and each desc's moving operand is ~3x wider,
  so the PE no longer pays the per-instruction floor.
- Extraction psum blocks are packed per n0 (only the union k-window),
  sig-blocks at a 512-f32 (one bank) stride; one desc writes 3 banks.
- Output tensor is bf16 (host casts to f32): halves the out-DMA bytes
  and the ob staging SBUF.
- Gram runs px-split with the slab holding one px phase at a time.
"""

import sys

for _p in ("/opt/trn_rl_repo",):
    if _p not in sys.path:
        sys.path.insert(0, _p)

import numpy as np

import concourse.bass as bass
import concourse.tile as tile
from concourse import bacc, mybir
from concourse.bass_utils import run_bass_kernel_spmd

F32 = mybir.dt.float32
BF16 = mybir.dt.bfloat16

B = 8
C = 256
H = 64
W = 96
HS, WS = H // 2, W // 2      # 32 x 48 sub-grid
R = 10                       # sub-displacement radius
P = 2 * R + 1                # 21
D = P * P                    # 441 output channels
WP = WS + R                  # 58: padded row pitch
XOFF = R
NP = WP * (HS - 1) + WS + 2 * R  # 1866 flat positions
NK = 15                      # 128-row m-chunks covering [0, 1866)
S0 = WP * R + XOFF           # 590: s_idx = (m - n) + S0
SIDX = WP * (P - 1) + P      # 1181 used diagonal offsets
NSIG = (SIDX + 127) // 128   # 10 sigma windows
KSTEP = NP + 128             # staircase step between k-chunk diagonals

WAVES = [(0, 3), (3, 6), (6, 9), (9, 10)]   # sigma waves
GROUPS = [(i * 16, (i + 1) * 16) for i in range(8)]   # n0 groups
BS = 64                      # psum block stride per n0 (f32), divides 512


def _t_list(n0):
    return [t for t in range(0, 12) if -128 < 128 * t + n0 - S0 < 128]


def _kmax(n0):
    return NK if n0 < NP - 128 * (NK - 1) else NK - 1


def _valid_win(sig, t, n0):
    lo, hi = max(0, t - sig), min(_kmax(n0), NK - (sig - t))
    return (lo, hi) if lo < hi else None


def _union_win(n0, s0, s1):
    """Union k-window over t in t_list and sig in [s0, s1)."""
    lo, hi = None, None
    for t in _t_list(n0):
        for sig in range(s0, s1):
            w = _valid_win(sig, t, n0)
            if w:
                lo = w[0] if lo is None else min(lo, w[0])
                hi = w[1] if hi is None else max(hi, w[1])
    return (lo, hi) if lo is not None else None


def _regime_runs(q0, q1):
    """n0 runs inside [q0,q1) where (t_list, kmax) is constant."""
    runs, start = [], q0
    for n0 in range(q0 + 1, q1 + 1):
        if n0 == q1 or (_t_list(n0), _kmax(n0)) != (_t_list(start), _kmax(start)):
            runs.append((start, n0))
            start = n0
    return runs


def _pads():
    lo = hi = 0
    for (s0, s1) in WAVES:
        for n0 in range(128):
            u = _union_win(n0, s0, s1)
            if not u:
                continue
            for t in _t_list(n0):
                lo = max(lo, t - s0 - u[0])
                hi = max(hi, (s1 - 1) - t + u[1] - 1 - (NK - 1))
    return max(lo, 0), max(hi, 0)


PADLO, PADHI = _pads()
NCH = PADLO + NK + PADHI     # total slab chunks


def _build_program(loop_n=None):
    nc = bacc.Bacc("TRN2", target_bir_lowering=False, debug=False)

    x1d = nc.dram_tensor("x1", [C, H, W], F32, kind="ExternalInput").ap()
    x2d = nc.dram_tensor("x2", [C, H, W], F32, kind="ExternalInput").ap()
    outd = nc.dram_tensor("out", [D, H, W], BF16, kind="ExternalOutput").ap()

    with tile.TileContext(nc) as tc:
        if loop_n is None:
            _corr_kernel(tc, outd, x1d, x2d)
        else:
            with tc.For_i(0, loop_n, 1):
                _corr_kernel(tc, outd, x1d, x2d)
    nc.compile()
    return nc


def _corr_kernel(tc, outd, x1d, x2d):
    from contextlib import ExitStack

    nc = tc.nc
    with ExitStack() as ctx:
        const_pool = ctx.enter_context(tc.tile_pool(name="const", bufs=1))
        stage_pool = ctx.enter_context(tc.tile_pool(name="stage", bufs=1))
        pack_pool = ctx.enter_context(tc.tile_pool(name="pack", bufs=1))
        slab_pool = ctx.enter_context(tc.tile_pool(name="slab", bufs=1))
        outb_pool = ctx.enter_context(tc.tile_pool(name="outb", bufs=1))
        psum_pool = ctx.enter_context(
            tc.tile_pool(name="psum", bufs=1, space="PSUM"))

        # --- shifted-identity strip: E[p, f] = 1 iff f - p == 128 ---
        ones = stage_pool.tile([128, 384], BF16, tag="stage")
        nc.vector.memset(ones[:, :], 1.0)
        E = const_pool.tile([128, 384], BF16)
        nc.gpsimd.affine_select(
            E[:, :], ones[:, :], pattern=[[1, 384]], base=-128,
            channel_multiplier=-1,
            compare_op=mybir.AluOpType.is_equal, fill=0.0)

        # --- persistent slab with zero pad chunks (memset once) ---
        slab = slab_pool.tile([128, NCH * NP], BF16, tag="slab")
        rp_s = slab.ap[0][0]
        if PADLO:
            nc.vector.memset(slab[:, 0:PADLO * NP], 0.0)
        if PADHI:
            nc.vector.memset(
                slab[:, (PADLO + NK) * NP:NCH * NP], 0.0)

        n_bal = 0

        def copy_op(dst, src):
            nonlocal n_bal
            if n_bal % 2:
                nc.scalar.copy(dst, src)
            else:
                nc.vector.tensor_copy(dst, src)
            n_bal += 1

        # ob tiles: one per sigma, px-interleaved cols 2n+px, bf16
        obs = []
        for sig in range(NSIG):
            obs.append(outb_pool.tile(
                [128, 2 * (NK * 128)], BF16, tag=f"ob{sig}",
                name=f"ob_{sig}"))

        for py in (0, 1):
            # ---- load + phase-pack inputs (both px) ----
            packed = {}
            for t_idx, src in ((0, x1d), (1, x2d)):
                for cc in (0, 1):
                    stg = stage_pool.tile([128, HS * W], F32, tag="stage",
                                          name=f"stg_{py}_{t_idx}_{cc}")
                    nc.sync.dma_start(
                        stg.rearrange("p (y x) -> p y x", x=W),
                        src[cc * 128:(cc + 1) * 128, py::2, :])
                    stg_v = stg.rearrange("p (y x) -> p y x", x=W)
                    for px in (0, 1):
                        pk = pack_pool.tile(
                            [128, NK * 128], BF16, tag=f"pk{t_idx}{cc}{px}",
                            name=f"pk_{py}_{t_idx}_{cc}_{px}")
                        nc.vector.memset(pk[:, :], 0.0)
                        dstv = bass.AP(pk.tensor, XOFF,
                                       [[pk.ap[0][0], 128], [WP, HS], [1, WS]])
                        copy_op(dstv, stg_v[:, :, px::2])
                        packed[(t_idx, cc, px)] = pk

            for px in (0, 1):
                # ---- Gram into padded slab (chunks PADLO..PADLO+NK) ----
                for k in range(NK):
                    for nb in range(4):
                        nbo = nb * 512
                        nbw = min(512, NP - nbo)
                        ps = psum_pool.tile([128, 512], F32,
                                            tag=f"g{(k * 4 + nb) % 2}",
                                            name=f"psg_{py}_{px}_{k}_{nb}")
                        for cc in (0, 1):
                            nc.tensor.matmul(
                                ps[:, 0:nbw],
                                lhsT=packed[(1, cc, px)][:, 128 * k:128 * (k + 1)],
                                rhs=packed[(0, cc, px)][:, nbo:nbo + nbw],
                                start=(cc == 0), stop=(cc == 1))
                        base = (PADLO + k) * NP + nbo
                        copy_op(slab[:, base:base + nbw], ps[:, 0:nbw])

                # ---- extraction: sig-wave x n0-group tiles ----
                # desc walk is k-outer / sig-inner (PE moving AP strides
                # must decrease outer->inner); psum block per n0 is a
                # contiguous [kw x ns] brick at a BS stride (bank-safe).
                for wi, (s0, s1) in enumerate(WAVES):
                    ns = s1 - s0
                    for gi, (q0, q1) in enumerate(GROUPS):
                        ps = psum_pool.tile(
                            [128, 16 * BS], F32, tag=f"ex{(gi + 8 * wi) % 3}",
                            name=f"pse_{py}_{px}_{wi}_{gi}")
                        pp = ps.ap[0][0]
                        # descs
                        for n0 in range(q0, q1):
                            u = _union_win(n0, s0, s1)
                            if not u:
                                continue
                            klo, khi = u
                            kw = khi - klo
                            ts = [t for t in _t_list(n0)
                                  if any(_valid_win(sig, t, n0)
                                         for sig in range(s0, s1))]
                            for ti, t in enumerate(ts):
                                a = 128 * t + n0 - S0 + 128
                                rhs = bass.AP(
                                    slab.tensor,
                                    NP * (PADLO + s0 - t + klo)
                                    + n0 + 128 * klo,
                                    [[rp_s, 128], [KSTEP, kw], [NP, ns]])
                                dst = bass.AP(
                                    ps.tensor, (n0 - q0) * BS,
                                    [[pp, 128], [1, kw * ns]])
                                nc.tensor.matmul(
                                    dst, lhsT=E[:, a:a + 128], rhs=rhs,
                                    start=(ti == 0), stop=(ti == len(ts) - 1))
                        # evac per (sig, regime-run)
                        for sl in range(ns):
                            sig = s0 + sl
                            for (r0, r1) in _regime_runs(q0, q1):
                                u = _union_win(r0, s0, s1)
                                if not u:
                                    continue
                                klo, khi = u
                                kw = khi - klo
                                src = bass.AP(
                                    ps.tensor, (r0 - q0) * BS + sl,
                                    [[pp, 128], [BS, r1 - r0], [ns, kw]])
                                dstv = bass.AP(
                                    obs[sig].tensor,
                                    2 * (r0 + 128 * klo) + px,
                                    [[obs[sig].ap[0][0], 128],
                                     [2, r1 - r0], [256, kw]])
                                copy_op(dstv, src)

            # ---- stale zero + DMA out per sigma ----
            for (s0, s1) in WAVES:
                for sig in range(s0, s1):
                    ob = obs[sig]
                    rp_o = ob.ap[0][0]
                    # zero ob columns never evac'd (out-of-range dy)
                    for (r0, r1) in _regime_runs(0, 128):
                        u = _union_win(r0, s0, s1)
                        km = _kmax(r0)
                        stales = []
                        if not u:
                            stales.append((0, km))
                        else:
                            if u[0] > 0:
                                stales.append((0, u[0]))
                            if u[1] < km:
                                stales.append((u[1], km))
                        for (ka, kb) in stales:
                            zap = bass.AP(
                                ob.tensor, 2 * (r0 + 128 * ka),
                                [[rp_o, 128], [2, r1 - r0],
                                 [256, kb - ka], [1, 2]])
                            nc.gpsimd.memset(zap, 0.0)
                    # d-contiguous row runs
                    sig_lo = 128 * sig
                    sig_hi = min(sig_lo + 128, SIDX)
                    for di in range(P):
                        r_lo = max(WP * di, sig_lo)
                        r_hi = min(WP * di + P, sig_hi)
                        if r_lo >= r_hi:
                            continue
                        dj0, dj1 = r_lo - WP * di, r_hi - WP * di
                        d0 = P * di + dj0
                        src = bass.AP(
                            ob.tensor,
                            (r_lo - sig_lo) * rp_o + 2 * XOFF,
                            [[rp_o, r_hi - r_lo], [2 * WP, HS], [1, W]])
                        nc.sync.dma_start(
                            outd[d0:d0 + (dj1 - dj0), py::2, :], src)


_PROGRAM = None


def _get_program():
    global _PROGRAM
    if _PROGRAM is None:
        _PROGRAM = _build_program()
    return _PROGRAM


def kernel(x1: np.ndarray, x2: np.ndarray) -> np.ndarray:
    x1 = np.ascontiguousarray(np.asarray(x1, dtype=np.float32))
    x2 = np.ascontiguousarray(np.asarray(x2, dtype=np.float32))
    assert x1.shape == (B, C, H, W) and x2.shape == (B, C, H, W)
    nc = _get_program()
    in_maps = [{"x1": x1[i], "x2": x2[i]} for i in range(B)]
    res = run_bass_kernel_spmd(nc, in_maps, core_ids=list(range(B)))
    return np.stack(
        [np.asarray(res.results[i]["out"]).astype(np.float32)
         for i in range(B)], axis=0)


if __name__ == "__main__":
    rng = np.random.default_rng(0)
    x1 = rng.standard_normal((B, C, H, W), dtype=np.float32)
    x2 = rng.standard_normal((B, C, H, W), dtype=np.float32)
    out = kernel(x1, x2)
    print(out.shape, out.dtype, float(np.abs(out).max()))
